# revision 1
# baseline (speedup 1.0000x reference)
# Multi-head attention kernel for Trainium2 (8 NeuronCores, SPMD).
#
# Problem (hardcoded): X[4, 2048, 1024], W_k/W_q/W_v/W_u[1024, 1024], b_u[1024]
#   K = (X @ W_k.T) * s ; Q = (X @ W_q.T) * s ; V = (X @ W_v.T) * s   (s = 1024**-0.25)
#   S = Q @ K.T per head (16 heads, head_dim 64); P = softmax(S); Y = P @ V
#   out = Y @ W_u.T + b_u
#
# Sharding: core c handles (batch c//2, query-half c%2). Each core computes
# K/V projections for its batch's full sequence (needed by every query) and Q
# for its query half; outputs are disjoint [1024, 1024] slices of the result,
# so the host-side unshard is a pure concatenation.
#
# Per-core data layout (everything oriented so the PE contracts on partitions):
#   X^T  [e, t]    from host (layout-only transform during sharding)
#   K^T  [e', t]   feature-major; head h lives on partitions (h%2)*64.. of tile h//2
#   Q^T  [e', q]   same
#   V    [t, h, 65] token-major, 65th column = ones (gives softmax denominator
#                   for free as row 64 of the P@V matmul output)
#   S_T  [tk, q]   scores transposed; exp is layout-agnostic and the AV matmul
#                   wants P with tk on partitions, so softmax needs no transposes
#   Y^T  [e, q]    AV output, normalized by 1/D broadcast (DMA bounce via DRAM)
#   out  [q, e']   token-major final projection (lhsT = Y^T tiles)

import numpy as np

import concourse.bacc as bacc
import concourse.mybir as mybir
import concourse.tile as tile
from concourse.bass_utils import run_bass_kernel_spmd

FP32 = mybir.dt.float32
BF16 = mybir.dt.bfloat16
AF = mybir.ActivationFunctionType

P = 128
E = 1024          # embedding dim
H = 16            # heads
S = 64            # head dim
ET = E // P       # 8 contraction tiles over e
SCALE = float(1024.0 ** -0.25)

N_CORES = 8


def build_nc(T, TQ):
    """Build + compile the per-core Bass module. T = full seq len on this core,
    TQ = query rows handled by this core."""
    assert T % P == 0 and TQ % P == 0 and E == H * S
    TT = T // P   # key tiles

    nc = bacc.Bacc("TRN2", target_bir_lowering=False, debug=False,
                   enable_asserts=False)

    # xt arrives rotated so that this core's TQ query tokens are columns
    # 0..TQ-1 (attention is permutation-invariant over the key/token axis,
    # so K/V built from the rotated order give identical query outputs)
    xt = nc.dram_tensor("xt", [E, T], FP32, kind="ExternalInput").ap()
    wkt = nc.dram_tensor("wkt", [E, E], FP32, kind="ExternalInput").ap()
    wqt = nc.dram_tensor("wqt", [E, E], FP32, kind="ExternalInput").ap()
    wvt = nc.dram_tensor("wvt", [E, E], FP32, kind="ExternalInput").ap()
    wut = nc.dram_tensor("wut", [E, E], FP32, kind="ExternalInput").ap()
    bu = nc.dram_tensor("bu", [1, E], FP32, kind="ExternalInput").ap()
    out = nc.dram_tensor("out", [TQ, E], FP32, kind="ExternalOutput").ap()

    with tile.TileContext(nc) as tc:
        _build_kernel(tc, nc, T, TQ, TT, xt, wkt, wqt, wvt, wut, bu, out)
    nc.compile()
    return nc


def _chunks(total, step):
    return [(o, min(step, total - o)) for o in range(0, total, step)]


def _build_kernel(tc, nc, T, TQ, TT, xt, wkt, wqt, wvt, wut, bu, out):
    """Single merged pipeline: V projection first (per t-tile), then per
    head-pair [K/Q projection -> QK scores -> exp -> incremental AV], then the
    output projection. AV accumulates into held PSUM banks as each exp tile
    lands, so P tiles are per (head, key-tile) and tiny — everything coexists
    in SBUF with no phase barriers, keeping the PE dense (HAM stays warm)."""
    with (
        tc.tile_pool(name="main", bufs=1) as mp,
        tc.tile_pool(name="psum", bufs=1, space="PSUM") as pspool,
        tc.tile_pool(name="dram", bufs=1, space="DRAM") as drampool,
    ):
        vv = mp.tile([P, TT, H, S + 1], BF16, tag="vv", name="vv")
        yt = mp.tile([P, ET, TQ], BF16, tag="yt", name="yt")
        bub = mp.tile([P, E], FP32, tag="bub", name="bub")
        nc.sync.dma_start(bub[:], bu.to_broadcast([P, E]))

        # --- X^T and W_v (scaled), DMA interleaved per k-tile so the first
        # projection matmuls can start after ~1.5 MB of input traffic
        xtb = mp.tile([P, ET, T], BF16, tag="xtb", name="xtb")
        wbv = mp.tile([P, ET, E], BF16, tag="wbv", name="wbv")
        for k in range(ET):
            ws = mp.tile([P, E], FP32, tag="ws", bufs=2, name=f"ws_v{k}")
            nc.sync.dma_start(ws[:], wvt[k * P:(k + 1) * P, :])
            nc.vector.tensor_scalar_mul(wbv[:, k, :], ws[:], SCALE)
            xs = mp.tile([P, T], FP32, tag="xs", bufs=2, name=f"xs{k}")
            nc.sync.dma_start(xs[:], xt[k * P:(k + 1) * P, :])
            nc.vector.tensor_copy(out=xtb[:, k, :], in_=xs[:])

        # --- per-pair K/Q weight load + projection, used with 1-pair lookahead
        def emit_proj(j):
            wkj = mp.tile([P, ET, P], BF16, tag="wkj", bufs=2, name=f"wk{j}")
            wqj = mp.tile([P, ET, P], BF16, tag="wqj", bufs=2, name=f"wq{j}")
            for wap, wb, uname in ((wkt, wkj, "k"), (wqt, wqj, "q")):
                wstg = mp.tile([P, ET, P], FP32, tag="wstg", bufs=2,
                               name=f"wstg_{uname}{j}")
                nc.sync.dma_start(
                    wstg[:],
                    wap[:, j * P:(j + 1) * P].rearrange(
                        "(ko p) m -> p ko m", p=P))
                nc.vector.tensor_scalar_mul(wb[:], wstg[:], SCALE)

            ktj = mp.tile([P, T], BF16, tag="ktj", bufs=2, name=f"kt{j}")
            for t0, tw in _chunks(T, 1024):
                ps = pspool.tile([P, 1024], FP32, tag="ps", bufs=2,
                                 name=f"psk{j}_{t0}")
                for n0, nw in _chunks(tw, 512):
                    for k in range(ET):
                        nc.tensor.matmul(
                            ps[:, n0:n0 + nw],
                            lhsT=wkj[:, k, :],
                            rhs=xtb[:, k, t0 + n0:t0 + n0 + nw],
                            start=(k == 0), stop=(k == ET - 1))
                nc.vector.tensor_copy(out=ktj[:, t0:t0 + tw], in_=ps[:, :tw])
            qtj = mp.tile([P, TQ], BF16, tag="qtj", bufs=2, name=f"qt{j}")
            for t0, tw in _chunks(TQ, 1024):
                ps = pspool.tile([P, 1024], FP32, tag="ps", bufs=2,
                                 name=f"psq{j}_{t0}")
                for n0, nw in _chunks(tw, 512):
                    for k in range(ET):
                        nc.tensor.matmul(
                            ps[:, n0:n0 + nw],
                            lhsT=wqj[:, k, :],
                            rhs=xtb[:, k, t0 + n0:t0 + n0 + nw],
                            start=(k == 0), stop=(k == ET - 1))
                nc.vector.tensor_copy(out=qtj[:, t0:t0 + tw], in_=ps[:, :tw])
            return ktj, qtj

        # pair 0's K/Q projection first: lets the score/exp stream start while
        # the V projection still occupies the PE
        kq = {0: emit_proj(0)}

        # --- V projection -> vv[t, h, 0:64] (token-major) + ones column
        for mt in range(TT):
            ps = pspool.tile([P, 1024], FP32, tag="ps", bufs=2,
                             name=f"psv{mt}")
            for n0, nw in _chunks(E, 512):
                for k in range(ET):
                    nc.tensor.matmul(
                        ps[:, n0:n0 + nw],
                        lhsT=xtb[:, k, mt * P:(mt + 1) * P],
                        rhs=wbv[:, k, n0:n0 + nw],
                        start=(k == 0), stop=(k == ET - 1))
            nc.vector.tensor_copy(out=vv[:, mt, :, 0:S],
                                  in_=ps[:].rearrange("p (h s) -> p h s", s=S))
            nc.vector.memset(vv[:, mt, :, S:S + 1], 1.0)

        # wut shares the wbv slot (wbv dead after V projection); DMA+cast it
        # early so the final projection never waits
        wub = mp.tile([P, ET, E], BF16, tag="wbv", name="wub")
        for k in range(ET):
            ws = mp.tile([P, E], FP32, tag="ws", bufs=2, name=f"ws_u{k}")
            nc.sync.dma_start(ws[:], wut[k * P:(k + 1) * P, :])
            nc.vector.tensor_copy(out=wub[:, k, :], in_=ws[:])

        # --- head pairs (K/Q projection software-pipelined one pair ahead)
        QTR = 4                      # key-tiles per AV burst
        for j in range(H // 2):
            ktj, qtj = kq.pop(j)

            # held AV accumulators: [parity][chunk] -> one PSUM bank each
            avs = {}
            for par in range(2):
                for ci, (c0, cw) in enumerate(_chunks(TQ, 512)):
                    avs[(par, ci)] = pspool.tile(
                        [P, 512], FP32, tag=f"av{par}_{ci}", bufs=1,
                        name=f"av{j}_{par}_{ci}")

            # QK -> exp fills quarter-sized P tiles; AV drains them in bursts
            nq = (TT + QTR - 1) // QTR
            for qi in range(nq):
                i0, i1 = qi * QTR, min((qi + 1) * QTR, TT)
                pts = [mp.tile([P, QTR, TQ], BF16, tag="pt", bufs=4,
                               name=f"p{j}_{qi}_{par}") for par in range(2)]
                for i in range(i0, i1):
                    for par in range(2):
                        lo = par * S
                        ps = pspool.tile([P, TQ], FP32, tag="ps", bufs=2,
                                         name=f"s{j}_{i}_{par}")
                        for c0, cw in _chunks(TQ, 512):
                            nc.tensor.matmul(
                                ps[:, c0:c0 + cw],
                                lhsT=ktj[lo:lo + S, i * P:(i + 1) * P],
                                rhs=qtj[lo:lo + S, c0:c0 + cw],
                                start=True, stop=True)
                        nc.scalar.activation(pts[par][:, i - i0, :], ps[:],
                                             AF.Exp)
                for par in range(2):
                    h = 2 * j + par
                    for ci, (c0, cw) in enumerate(_chunks(TQ, 512)):
                        for i in range(i0, i1):
                            nc.tensor.matmul(
                                avs[(par, ci)][0:S + 1, :cw],
                                lhsT=vv[:, i, h, :],
                                rhs=pts[par][:, i - i0, c0:c0 + cw],
                                start=(i == 0), stop=(i == TT - 1))
                if qi == 0 and j + 1 < H // 2:
                    # prefetch next pair's K/Q projection into the PE's slack
                    kq[j + 1] = emit_proj(j + 1)

            # evict AV banks immediately via DMA (frees PSUM for the next
            # pair), then normalize: 1/D, partition-broadcast via DRAM bounce,
            # multiply, land in yt (odd head partition-shifted by DMA)
            for par in range(2):
                for ci, (c0, cw) in enumerate(_chunks(TQ, 512)):
                    av = avs[(par, ci)]
                    yraw = mp.tile([S + 1, 512], FP32, tag="yraw", bufs=3,
                                   name=f"yraw{j}_{par}_{ci}")
                    nc.vector.tensor_copy(out=yraw[:, :cw],
                                          in_=av[0:S + 1, :cw])
                    dr = mp.tile([S + 1, 512], FP32, tag="dr", bufs=1,
                                 name=f"dr{j}_{par}_{ci}")
                    nc.vector.reciprocal(out=dr[S:S + 1, :cw],
                                         in_=yraw[S:S + 1, :cw])
                    db = drampool.tile([1, 512], FP32, tag="db", bufs=8,
                                       name=f"db{j}_{par}_{ci}")
                    nc.sync.dma_start(db[:, :cw], dr[S:S + 1, :cw])
                    rbc = mp.tile([S, 512], FP32, tag="rbc", bufs=2,
                                  name=f"rbc{j}_{par}_{ci}")
                    nc.sync.dma_start(rbc[:, :cw],
                                      db[:, :cw].to_broadcast([S, cw]))
                    if par == 0:
                        nc.vector.tensor_mul(out=yt[0:S, j, c0:c0 + cw],
                                             in0=yraw[0:S, :cw],
                                             in1=rbc[:, :cw])
                    else:
                        tmp = mp.tile([S, 512], BF16, tag="tmp", bufs=2,
                                      name=f"tmp{j}_{ci}")
                        nc.vector.tensor_mul(out=tmp[:, :cw],
                                             in0=yraw[0:S, :cw],
                                             in1=rbc[:, :cw])
                        nc.sync.dma_start(yt[S:P, j, c0:c0 + cw],
                                          tmp[:, :cw])

        # --- output projection out[q, e'] = Y^T.T @ W_u^T + b_u
        for m in range(TQ // P):
            ps = pspool.tile([P, 1024], FP32, tag="ps", bufs=2,
                             name=f"o{m}")
            for n0, nw in _chunks(E, 512):
                for k in range(ET):
                    nc.tensor.matmul(
                        ps[:, n0:n0 + nw],
                        lhsT=yt[:, k, m * P:(m + 1) * P],
                        rhs=wub[:, k, n0:n0 + nw],
                        start=(k == 0), stop=(k == ET - 1))
            ot = mp.tile([P, E], FP32, tag="ot", bufs=2, name=f"ot{m}")
            nc.vector.tensor_add(out=ot[:], in0=ps[:], in1=bub[:])
            nc.sync.dma_start(out[m * P:(m + 1) * P, :], ot[:])


_NC_CACHE = {}


def _get_nc(T, TQ):
    key = (T, TQ)
    if key not in _NC_CACHE:
        _NC_CACHE[key] = build_nc(T, TQ)
    return _NC_CACHE[key]


def make_in_maps(X, W_k, W_q, W_v, W_u, b_u):
    X = np.asarray(X, np.float32)
    b, t, e = X.shape
    tq = t // 2
    wk_t = np.ascontiguousarray(np.asarray(W_k, np.float32).T)
    wq_t = np.ascontiguousarray(np.asarray(W_q, np.float32).T)
    wv_t = np.ascontiguousarray(np.asarray(W_v, np.float32).T)
    wu_t = np.ascontiguousarray(np.asarray(W_u, np.float32).T)
    bu2 = np.ascontiguousarray(np.asarray(b_u, np.float32).reshape(1, e))
    in_maps = []
    for c in range(N_CORES):
        bi, qo = c // 2, (c % 2) * tq
        xt_np = X[bi].T
        # rotate so this core's query tokens are columns 0..tq-1
        xt_np = np.ascontiguousarray(
            np.concatenate([xt_np[:, qo:], xt_np[:, :qo]], axis=1))
        in_maps.append({
            "xt": xt_np,
            "wkt": wk_t, "wqt": wq_t, "wvt": wv_t, "wut": wu_t,
            "bu": bu2,
        })
    return in_maps


def run(inputs, trace=False, **kwargs):
    """Run on hardware; returns (full output, BassKernelResults)."""
    X = np.asarray(inputs["X"], np.float32)
    b, t, e = X.shape
    tq = t // 2
    nc = _get_nc(t, tq)
    in_maps = make_in_maps(X, inputs["W_k"], inputs["W_q"], inputs["W_v"],
                           inputs["W_u"], inputs["b_u"])
    res = run_bass_kernel_spmd(nc, in_maps, core_ids=list(range(N_CORES)),
                               trace=trace, **kwargs)
    full = np.empty((b, t, e), np.float32)
    for c in range(N_CORES):
        full[c // 2, (c % 2) * tq:(c % 2) * tq + tq, :] = res.results[c]["out"]
    return full, res


def kernel(**inputs):
    full, _ = run(inputs)
    return full



# revision 7
# speedup vs baseline: 1.1199x; 1.1199x over previous
# Multi-head attention kernel for Trainium2 (8 NeuronCores, SPMD).
#
# Problem (hardcoded): X[4, 2048, 1024], W_k/W_q/W_v/W_u[1024, 1024], b_u[1024]
#   K = (X @ W_k.T) * s ; Q = (X @ W_q.T) * s ; V = (X @ W_v.T) * s   (s = 1024**-0.25)
#   S = Q @ K.T per head (16 heads, head_dim 64); P = softmax(S); Y = P @ V
#   out = Y @ W_u.T + b_u
#
# Sharding: core c handles (batch c//2, head-group c%2) -- 8 of the 16 heads.
# Each core computes K/Q/V projections only for its own 8 heads (512 of the
# 1024 features), runs attention for those heads over the full sequence, and
# produces a PARTIAL output projection out_c = (Y_c / D_c) @ W_u.T[gc] + b_u/2.
# The host unshard sums the two partial outputs per batch (pure reduction).
#
# Per-core layout (PE always contracts on partitions):
#   X^T   [e, t]        bf16 from host
#   K^T   [128, j, t]   pair-major: pair j's heads at partitions 0-63 / 64-127
#   Q^T   [128, j, t]   same; lets the two heads of a pair run as CONCURRENT
#                       row-tiled score matmuls (K=64 each, tile (0,0)+(64,0))
#   V     [t, i, h, 65] token-major, 65th column = ones (softmax denominator
#                       arrives free as row 64 of the P@V accumulation)
#   S^T   [tk, q]       scores transposed; exp is layout-agnostic and AV wants
#                       P with keys on partitions
#   Y^T   [128, kt, q]  normalized AV output, bf16, feeds the out-projection
#
# Scale s is folded into the host-side weight slices; all inputs arrive bf16.

import numpy as np
import ml_dtypes

import concourse.bacc as bacc
import concourse.mybir as mybir
import concourse.tile as tile
from concourse.bass_utils import run_bass_kernel_spmd

FP32 = mybir.dt.float32
BF16 = mybir.dt.bfloat16
AF = mybir.ActivationFunctionType

P = 128
E = 1024          # embedding dim
F = 512           # features per core (8 heads x 64)
H = 8             # heads per core
S = 64            # head dim
ET = E // P       # 8 contraction tiles over e
FT = F // P       # 4 feature tiles (= head pairs)
T = 2048          # sequence length
TT = T // P       # 16 key tiles
NQH = 4           # query quarters
QW = T // NQH     # 512 queries per quarter
SCALE = float(1024.0 ** -0.25)

N_CORES = 8


def build_nc():
    nc = bacc.Bacc("TRN2", target_bir_lowering=False, debug=False,
                   enable_asserts=False)

    xt = nc.dram_tensor("xt", [E, T], BF16, kind="ExternalInput").ap()
    wk = nc.dram_tensor("wk", [E, F], BF16, kind="ExternalInput").ap()
    wq = nc.dram_tensor("wq", [E, F], BF16, kind="ExternalInput").ap()
    wv = nc.dram_tensor("wv", [E, F], BF16, kind="ExternalInput").ap()
    wu = nc.dram_tensor("wu", [F, E], BF16, kind="ExternalInput").ap()
    bu = nc.dram_tensor("bu", [1, E], FP32, kind="ExternalInput").ap()
    out = nc.dram_tensor("out", [T, E], FP32, kind="ExternalOutput").ap()

    with tile.TileContext(nc) as tc:
        _build_kernel(tc, nc, xt, wk, wq, wv, wu, bu, out)
    nc.compile()
    return nc


def _build_kernel(tc, nc, xt, wk, wq, wv, wu, bu, out):
    with (
        tc.tile_pool(name="main", bufs=1) as mp,
        tc.tile_pool(name="psum", bufs=1, space="PSUM") as pspool,
        tc.tile_pool(name="dram", bufs=1, space="DRAM") as drampool,
    ):
        bub = mp.tile([P, E], FP32, tag="bub", name="bub")
        nc.sync.dma_start(bub[:], bu.to_broadcast([P, E]))

        # inputs, bf16, tiled for contraction on partitions
        xtb = mp.tile([P, ET, T], BF16, tag="xtb", name="xtb")
        wkb = mp.tile([P, ET, F], BF16, tag="wkb", name="wkb")
        wqb = mp.tile([P, ET, F], BF16, tag="wqb", name="wqb")
        wvb = mp.tile([P, ET, F], BF16, tag="wvb", name="wvb")
        for k in range(ET):
            nc.sync.dma_start(xtb[:, k, :], xt[k * P:(k + 1) * P, :])
            nc.sync.dma_start(wkb[:, k, :], wk[k * P:(k + 1) * P, :])
            nc.sync.dma_start(wqb[:, k, :], wq[k * P:(k + 1) * P, :])
            nc.sync.dma_start(wvb[:, k, :], wv[k * P:(k + 1) * P, :])

        ktb = mp.tile([P, FT, T], BF16, tag="ktb", name="ktb")
        qtb = mp.tile([P, FT, T], BF16, tag="qtb", name="qtb")
        vv = mp.tile([P, TT, H, S + 1], BF16, tag="vv", name="vv")
        yt = mp.tile([P, FT, T], BF16, tag="yt", name="yt")

        def kq_chunk(j, wb, dst, c0):
            """One 512-column chunk of the K or Q projection for pair j."""
            ps = pspool.tile([P, 1024], FP32, tag="ps", bufs=2,
                             name=f"pskq{j}_{id(wb)}_{c0}")
            for k in range(ET):
                nc.tensor.matmul(
                    ps[:, 0:512],
                    lhsT=wb[:, k, j * P:(j + 1) * P],
                    rhs=xtb[:, k, c0:c0 + 512],
                    start=(k == 0), stop=(k == ET - 1))
            nc.vector.tensor_copy(out=dst[:, j, c0:c0 + 512], in_=ps[:, 0:512])

        def v_group(mt):
            """V projection for token tile mt -> vv[:, mt] + ones column."""
            ps = pspool.tile([P, 1024], FP32, tag="ps", bufs=2,
                             name=f"psv{mt}")
            for k in range(ET):
                nc.tensor.matmul(
                    ps[:, 0:512],
                    lhsT=xtb[:, k, mt * P:(mt + 1) * P],
                    rhs=wvb[:, k, :],
                    start=(k == 0), stop=(k == ET - 1))
            nc.vector.tensor_copy(
                out=vv[:, mt, :, 0:S],
                in_=ps[:, 0:512].rearrange("p (h s) -> p h s", s=S))
            nc.vector.memset(vv[:, mt, :, S:S + 1], 1.0)

        # K/Q projection of pair 0 first (fills the score pipeline), then a
        # head start on V so AV never outruns the V projection.
        for c0 in range(0, T, 512):
            kq_chunk(0, wkb, ktb, c0)
            kq_chunk(0, wqb, qtb, c0)
        for mt in range(3):
            v_group(mt)

        # wu shares wvb's slot (dead after V projection completes)
        wub = mp.tile([P, FT, E], BF16, tag="wvb", name="wub")

        def load_wu():
            nc.sync.dma_start(
                wub[:], wu.rearrange("(kt p) e -> p kt e", p=P))

        # per-(pair, slot) filler work, spread into the attention loop's PE
        # slack so the scalar engine (exp) stays the critical path. V tiles
        # must land at >= 1/slot during pair 0's first quarter (AV consumes
        # V(i) at slot i), so they occupy slots 0..12 back-to-back.
        sched = {j: {} for j in range(FT)}
        for idx, mt in enumerate(range(3, TT)):
            sched[0].setdefault(idx, []).append(lambda mt=mt: v_group(mt))

        def kq_closures(nj):
            return [
                (lambda nj=nj, wb=wb, dst=dst, c0=c0:
                 kq_chunk(nj, wb, dst, c0))
                for wb, dst in ((wkb, ktb), (wqb, qtb))
                for c0 in range(0, T, 512)
            ]

        rest0 = kq_closures(1) + [load_wu]
        for idx, fn in enumerate(rest0):
            s = 13 + (NQH * TT - 13) * idx // len(rest0)
            sched[0].setdefault(s, []).append(fn)
        for j in (1, 2):
            lst = kq_closures(j + 1)
            for idx, fn in enumerate(lst):
                sched[j].setdefault(NQH * TT * idx // len(lst), []).append(fn)

        # epilogue state: unnormalized Y (bf16) + denominators staged in DRAM
        yraws = {}
        dramd = {qh: drampool.tile([H, QW], BF16, tag=f"dD{qh}", bufs=1,
                                   name=f"dramD{qh}")
                 for qh in range(NQH)}

        def epilogue(qh):
            """Batched reciprocal of this quarter's 8 denominators, broadcast
            via DRAM bounce, normalize into yt, then the output projection."""
            q0 = qh * QW
            dsb = mp.tile([P, QW], BF16, tag="dsb", bufs=2, name=f"dsb{qh}")
            nc.sync.dma_start(dsb[0:H, :], dramd[qh][:])
            rcb = mp.tile([P, QW], FP32, tag="rcb", bufs=2, name=f"rcb{qh}")
            nc.vector.reciprocal(out=rcb[0:H, :], in_=dsb[0:H, :])
            dramr = drampool.tile([H, QW], FP32, tag="dR", bufs=2,
                                  name=f"dramR{qh}")
            nc.sync.dma_start(dramr[:], rcb[0:H, :])
            for j in range(FT):
                for par in range(2):
                    h = 2 * j + par
                    rbc = mp.tile([S, QW], FP32, tag="rbc", bufs=4,
                                  name=f"rbc{qh}_{h}")
                    nc.sync.dma_start(
                        rbc[:], dramr[h:h + 1, :].to_broadcast([S, QW]))
                    yraw = yraws.pop((qh, j, par))
                    if par == 0:
                        nc.vector.tensor_mul(out=yt[0:S, j, q0:q0 + QW],
                                             in0=yraw[0:S, :], in1=rbc[:])
                    else:
                        tmp = mp.tile([S, QW], BF16, tag="tmp", bufs=2,
                                      name=f"tmp{qh}_{h}")
                        nc.vector.tensor_mul(out=tmp[:], in0=yraw[0:S, :],
                                             in1=rbc[:])
                        nc.sync.dma_start(yt[S:P, j, q0:q0 + QW], tmp[:])
            for m in range(QW // P):
                ps = pspool.tile([P, 1024], FP32, tag="ps", bufs=2,
                                 name=f"pso{qh}_{m}")
                mc = q0 + m * P
                for n0 in (0, 512):
                    for kt in range(FT):
                        nc.tensor.matmul(
                            ps[:, n0:n0 + 512],
                            lhsT=yt[:, kt, mc:mc + P],
                            rhs=wub[:, kt, n0:n0 + 512],
                            start=(kt == 0), stop=(kt == FT - 1))
                ot = mp.tile([P, E], FP32, tag="ot", bufs=2,
                             name=f"ot{qh}_{m}")
                nc.vector.tensor_add(out=ot[:], in0=ps[:], in1=bub[:])
                nc.sync.dma_start(out[mc:mc + P, :], ot[:])

        # --- attention: pair-outer, query-quarter inner ---
        for j in range(FT):
            fill = sched[j]
            slot = 0
            for qh in range(NQH):
                q0 = qh * QW
                avA = pspool.tile([P, QW], FP32, tag="avA", bufs=2,
                                  name=f"av{j}_{qh}_0")
                avB = pspool.tile([P, QW], FP32, tag="avB", bufs=2,
                                  name=f"av{j}_{qh}_1")
                for i in range(TT):
                    ps = pspool.tile([P, 1024], FP32, tag="ps", bufs=2,
                                     name=f"s{j}_{qh}_{i}")
                    # two heads of the pair: concurrent row-tiled matmuls
                    nc.tensor.matmul(
                        ps[:, 0:512],
                        lhsT=ktb[0:S, j, i * P:(i + 1) * P],
                        rhs=qtb[0:S, j, q0:q0 + QW],
                        start=True, stop=True)
                    nc.tensor.matmul(
                        ps[:, 512:1024],
                        lhsT=ktb[S:P, j, i * P:(i + 1) * P],
                        rhs=qtb[S:P, j, q0:q0 + QW],
                        start=True, stop=True)
                    pts = mp.tile([P, 1024], BF16, tag="pt", bufs=3,
                                  name=f"p{j}_{qh}_{i}")
                    nc.scalar.activation(pts[:], ps[:], AF.Exp)
                    nc.tensor.matmul(
                        avA[0:S + 1, :],
                        lhsT=vv[:, i, 2 * j, :],
                        rhs=pts[:, 0:512],
                        start=(i == 0), stop=(i == TT - 1))
                    nc.tensor.matmul(
                        avB[0:S + 1, :],
                        lhsT=vv[:, i, 2 * j + 1, :],
                        rhs=pts[:, 512:1024],
                        start=(i == 0), stop=(i == TT - 1))
                    # spread filler projections evenly over this pair's slots
                    for fn in fill.get(slot, ()):
                        fn()
                    slot += 1
                # drain AV into bf16 staging; denominator row -> DRAM
                for par, av in ((0, avA), (1, avB)):
                    yraw = mp.tile([P, QW], BF16, tag="yraw", bufs=34,
                                   name=f"yraw{j}_{qh}_{par}")
                    nc.vector.tensor_copy(out=yraw[0:S + 1, :],
                                          in_=av[0:S + 1, :])
                    nc.sync.dma_start(dramd[qh][2 * j + par:2 * j + par + 1, :],
                                      yraw[S:S + 1, :])
                    yraws[(qh, j, par)] = yraw
            if j == FT - 1:
                for qh in range(NQH):
                    epilogue(qh)
            elif j == FT - 2:
                pass  # epilogues all run after the last pair


_NC = None


def _get_nc():
    global _NC
    if _NC is None:
        _NC = build_nc()
    return _NC


def make_in_maps(X, W_k, W_q, W_v, W_u, b_u):
    bf16 = ml_dtypes.bfloat16
    X = np.asarray(X, np.float32)
    b = X.shape[0]
    wkt = (np.asarray(W_k, np.float32).T * SCALE).astype(bf16)
    wqt = (np.asarray(W_q, np.float32).T * SCALE).astype(bf16)
    wvt = (np.asarray(W_v, np.float32).T * SCALE).astype(bf16)
    wut = np.ascontiguousarray(np.asarray(W_u, np.float32).T).astype(bf16)
    bu2 = np.ascontiguousarray(
        (np.asarray(b_u, np.float32) * 0.5).reshape(1, E))
    xts = [np.ascontiguousarray(X[bi].T).astype(bf16) for bi in range(b)]
    in_maps = []
    for c in range(N_CORES):
        bi, pg = c // 2, c % 2
        f0 = pg * F
        in_maps.append({
            "xt": xts[bi],
            "wk": np.ascontiguousarray(wkt[:, f0:f0 + F]),
            "wq": np.ascontiguousarray(wqt[:, f0:f0 + F]),
            "wv": np.ascontiguousarray(wvt[:, f0:f0 + F]),
            "wu": np.ascontiguousarray(wut[f0:f0 + F, :]),
            "bu": bu2,
        })
    return in_maps


def run(inputs, trace=False, **kwargs):
    """Run on hardware; returns (full output, BassKernelResults)."""
    X = np.asarray(inputs["X"], np.float32)
    b, t, e = X.shape
    nc = _get_nc()
    in_maps = make_in_maps(X, inputs["W_k"], inputs["W_q"], inputs["W_v"],
                           inputs["W_u"], inputs["b_u"])
    res = run_bass_kernel_spmd(nc, in_maps, core_ids=list(range(N_CORES)),
                               trace=trace, **kwargs)
    full = np.empty((b, t, e), np.float32)
    for bi in range(b):
        full[bi] = res.results[2 * bi]["out"] + res.results[2 * bi + 1]["out"]
    return full, res


def kernel(**inputs):
    full, _ = run(inputs)
    return full


# revision 11
# speedup vs baseline: 1.1846x; 1.0577x over previous
# Multi-head attention kernel for Trainium2 (8 NeuronCores, SPMD).
#
# Problem (hardcoded): X[4, 2048, 1024], W_k/W_q/W_v/W_u[1024, 1024], b_u[1024]
#   K = (X @ W_k.T) * s ; Q = (X @ W_q.T) * s ; V = (X @ W_v.T) * s   (s = 1024**-0.25)
#   S = Q @ K.T per head (16 heads, head_dim 64); P = softmax(S); Y = P @ V
#   out = Y @ W_u.T + b_u
#
# Sharding: core c handles (batch c//2, head-group c%2) -- 8 of the 16 heads.
# Each core computes K/Q/V projections only for its own 8 heads (512 of the
# 1024 features), runs attention for those heads over the full sequence, and
# produces a PARTIAL output projection out_c = (Y_c / D_c) @ W_u.T[gc] + b_u/2.
# The host unshard sums the two partial outputs per batch (pure reduction).
#
# Per-core layout (PE always contracts on partitions):
#   X^T   [e, t]        bf16 from host
#   K^T   [128, j, t]   pair-major: pair j's heads at partitions 0-63 / 64-127
#   Q^T   [128, j, t]   same; lets the two heads of a pair run as CONCURRENT
#                       row-tiled score matmuls (K=64 each, tile (0,0)+(64,0))
#   V     [t, i, h, 65] token-major, 65th column = ones (softmax denominator
#                       arrives free as row 64 of the P@V accumulation)
#   S^T   [tk, q]       scores transposed; exp is layout-agnostic and AV wants
#                       P with keys on partitions
#   Y^T   [128, kt, q]  normalized AV output, bf16, feeds the out-projection
#
# Scale s is folded into the host-side weight slices; all inputs arrive bf16.

import numpy as np
import ml_dtypes

import concourse.bacc as bacc
import concourse.mybir as mybir
import concourse.tile as tile
from concourse.bass_utils import run_bass_kernel_spmd

FP32 = mybir.dt.float32
BF16 = mybir.dt.bfloat16
AF = mybir.ActivationFunctionType

P = 128
E = 1024          # embedding dim
F = 512           # features per core (8 heads x 64)
H = 8             # heads per core
S = 64            # head dim
ET = E // P       # 8 contraction tiles over e
FT = F // P       # 4 feature tiles (= head pairs)
T = 2048          # sequence length
TT = T // P       # 16 key tiles
NQH = 4           # query quarters
QW = T // NQH     # 512 queries per quarter
SCALE = float(1024.0 ** -0.25)

N_CORES = 8


def build_nc():
    nc = bacc.Bacc("TRN2", target_bir_lowering=False, debug=False,
                   enable_asserts=False)

    xt = nc.dram_tensor("xt", [E, T], BF16, kind="ExternalInput").ap()
    wk = nc.dram_tensor("wk", [E, F], BF16, kind="ExternalInput").ap()
    wq = nc.dram_tensor("wq", [E, F], BF16, kind="ExternalInput").ap()
    wv = nc.dram_tensor("wv", [E, F], BF16, kind="ExternalInput").ap()
    wu = nc.dram_tensor("wu", [F, E], BF16, kind="ExternalInput").ap()
    bu = nc.dram_tensor("bu", [1, E], FP32, kind="ExternalInput").ap()
    out = nc.dram_tensor("out", [T, E], FP32, kind="ExternalOutput").ap()

    with tile.TileContext(nc) as tc:
        _build_kernel(tc, nc, xt, wk, wq, wv, wu, bu, out)
    nc.compile()
    return nc


def _build_kernel(tc, nc, xt, wk, wq, wv, wu, bu, out):
    with (
        tc.tile_pool(name="main", bufs=1) as mp,
        tc.tile_pool(name="psum", bufs=1, space="PSUM") as pspool,
        tc.tile_pool(name="dram", bufs=1, space="DRAM") as drampool,
    ):
        bub = mp.tile([P, E], FP32, tag="bub", name="bub")
        nc.sync.dma_start(bub[:], bu.to_broadcast([P, E]))

        # inputs, bf16, tiled for contraction on partitions
        xtb = mp.tile([P, ET, T], BF16, tag="xtb", name="xtb")
        wkb = mp.tile([P, ET, F], BF16, tag="wkb", name="wkb")
        wqb = mp.tile([P, ET, F], BF16, tag="wqb", name="wqb")
        wvb = mp.tile([P, ET, F], BF16, tag="wvb", name="wvb")
        for k in range(ET):
            nc.sync.dma_start(xtb[:, k, :], xt[k * P:(k + 1) * P, :])
            nc.sync.dma_start(wkb[:, k, :], wk[k * P:(k + 1) * P, :])
            nc.sync.dma_start(wqb[:, k, :], wq[k * P:(k + 1) * P, :])
            nc.sync.dma_start(wvb[:, k, :], wv[k * P:(k + 1) * P, :])

        ktb = mp.tile([P, FT, T], BF16, tag="ktb", name="ktb")
        qtb = mp.tile([P, FT, T], BF16, tag="qtb", name="qtb")
        vv = mp.tile([P, TT, H, S + 1], BF16, tag="vv", name="vv")
        yt = mp.tile([P, FT, T], BF16, tag="yt", name="yt")

        def kq_piece(j, wb, dst, c0, half, state):
            """Half of one 512-column K/Q projection chunk (4 of 8 k-tiles);
            the accumulation group spans both pieces so a piece fits in the
            attention loop's per-slot PE slack."""
            if half == 0:
                state[(j, id(wb), c0)] = pspool.tile(
                    [P, 1024], FP32, tag="ps", bufs=2,
                    name=f"pskq{j}_{id(wb)}_{c0}")
            ps = state[(j, id(wb), c0)]
            for k in range(half * 4, half * 4 + 4):
                nc.tensor.matmul(
                    ps[:, 0:512],
                    lhsT=wb[:, k, j * P:(j + 1) * P],
                    rhs=xtb[:, k, c0:c0 + 512],
                    start=(k == 0), stop=(k == ET - 1))
            if half == 1:
                del state[(j, id(wb), c0)]
                nc.vector.tensor_copy(out=dst[:, j, c0:c0 + 512],
                                      in_=ps[:, 0:512])

        def v_group(mt):
            """V projection for token tile mt -> vv[:, mt] + ones column."""
            ps = pspool.tile([P, 1024], FP32, tag="ps", bufs=2,
                             name=f"psv{mt}")
            for k in range(ET):
                nc.tensor.matmul(
                    ps[:, 0:512],
                    lhsT=xtb[:, k, mt * P:(mt + 1) * P],
                    rhs=wvb[:, k, :],
                    start=(k == 0), stop=(k == ET - 1))
            nc.vector.tensor_copy(
                out=vv[:, mt, :, 0:S],
                in_=ps[:, 0:512].rearrange("p (h s) -> p h s", s=S))
            nc.vector.memset(vv[:, mt, :, S:S + 1], 1.0)

        # K/Q projection of pair 0 first (fills the score pipeline), then a
        # head start on V so AV never outruns the V projection.
        _st = {}
        for c0 in range(0, T, 512):
            for wb, dst in ((wkb, ktb), (wqb, qtb)):
                kq_piece(0, wb, dst, c0, 0, _st)
                kq_piece(0, wb, dst, c0, 1, _st)
        for mt in range(3):
            v_group(mt)

        # wu shares wvb's slot (dead after V projection completes)
        wub = mp.tile([P, FT, E], BF16, tag="wvb", name="wub")

        def load_wu():
            nc.sync.dma_start(
                wub[:], wu.rearrange("(kt p) e -> p kt e", p=P))

        # per-(pair, slot) filler work, spread into the attention loop's PE
        # slack so the scalar engine (exp) stays the critical path. V tiles
        # must land at >= 1/slot during pair 0's first quarter (AV consumes
        # V(i) at slot i), so they occupy slots 0..12 back-to-back.
        sched = {j: {} for j in range(FT)}
        for idx, mt in enumerate(range(3, TT)):
            sched[0].setdefault(idx, []).append(lambda mt=mt: v_group(mt))

        kq_state = {}

        def kq_closures(nj):
            return [
                (lambda nj=nj, wb=wb, dst=dst, c0=c0, half=half:
                 kq_piece(nj, wb, dst, c0, half, kq_state))
                for wb, dst in ((wkb, ktb), (wqb, qtb))
                for c0 in range(0, T, 512)
                for half in (0, 1)
            ]

        rest0 = kq_closures(1) + [load_wu]
        for idx, fn in enumerate(rest0):
            s = 13 + (NQH * TT - 13) * idx // len(rest0)
            sched[0].setdefault(s, []).append(fn)
        for j in (1, 2):
            lst = kq_closures(j + 1)
            for idx, fn in enumerate(lst):
                sched[j].setdefault(NQH * TT * idx // len(lst), []).append(fn)

        # epilogue state: unnormalized Y (bf16) + denominators staged in DRAM
        yraws = {}
        dramd = {qh: drampool.tile([H, QW], BF16, tag=f"dD{qh}", bufs=1,
                                   name=f"dramD{qh}")
                 for qh in range(NQH)}

        def epilogue(qh):
            """Batched reciprocal of this quarter's 8 denominators, broadcast
            via DRAM bounce, normalize into yt, then the output projection."""
            q0 = qh * QW
            dsb = mp.tile([P, QW], BF16, tag="dsb", bufs=2, name=f"dsb{qh}")
            nc.sync.dma_start(dsb[0:H, :], dramd[qh][:])
            rcb = mp.tile([P, QW], FP32, tag="rcb", bufs=2, name=f"rcb{qh}")
            nc.vector.reciprocal(out=rcb[0:H, :], in_=dsb[0:H, :])
            dramr = drampool.tile([H, QW], FP32, tag="dR", bufs=2,
                                  name=f"dramR{qh}")
            nc.sync.dma_start(dramr[:], rcb[0:H, :])
            for j in range(FT):
                for par in range(2):
                    h = 2 * j + par
                    rbc = mp.tile([S, QW], FP32, tag="rbc", bufs=4,
                                  name=f"rbc{qh}_{h}")
                    nc.sync.dma_start(
                        rbc[:], dramr[h:h + 1, :].to_broadcast([S, QW]))
                    yraw = yraws.pop((qh, j, par))
                    if par == 0:
                        nc.vector.tensor_mul(out=yt[0:S, j, q0:q0 + QW],
                                             in0=yraw[0:S, :], in1=rbc[:])
                    else:
                        tmp = mp.tile([S, QW], BF16, tag="tmp", bufs=2,
                                      name=f"tmp{qh}_{h}")
                        nc.vector.tensor_mul(out=tmp[:], in0=yraw[0:S, :],
                                             in1=rbc[:])
                        nc.sync.dma_start(yt[S:P, j, q0:q0 + QW], tmp[:])
            for m in range(QW // P):
                ps = pspool.tile([P, 1024], FP32, tag="ps", bufs=2,
                                 name=f"pso{qh}_{m}")
                mc = q0 + m * P
                for n0 in (0, 512):
                    for kt in range(FT):
                        nc.tensor.matmul(
                            ps[:, n0:n0 + 512],
                            lhsT=yt[:, kt, mc:mc + P],
                            rhs=wub[:, kt, n0:n0 + 512],
                            start=(kt == 0), stop=(kt == FT - 1))
                ot = mp.tile([P, E], FP32, tag="ot", bufs=2,
                             name=f"ot{qh}_{m}")
                nc.vector.tensor_add(out=ot[:], in0=ps[:], in1=bub[:])
                nc.sync.dma_start(out[mc:mc + P, :], ot[:])

        # --- attention: pair-outer, query-quarter inner ---
        for j in range(FT):
            fill = sched[j]
            slot = 0
            for qh in range(NQH):
                q0 = qh * QW
                avA = pspool.tile([P, QW], FP32, tag="avA", bufs=2,
                                  name=f"av{j}_{qh}_0")
                avB = pspool.tile([P, QW], FP32, tag="avB", bufs=2,
                                  name=f"av{j}_{qh}_1")
                def emit_av(i, pts):
                    nc.tensor.matmul(
                        avA[0:S + 1, :],
                        lhsT=vv[:, i, 2 * j, :],
                        rhs=pts[:, 0:512],
                        start=(i == 0), stop=(i == TT - 1))
                    nc.tensor.matmul(
                        avB[0:S + 1, :],
                        lhsT=vv[:, i, 2 * j + 1, :],
                        rhs=pts[:, 512:1024],
                        start=(i == 0), stop=(i == TT - 1))

                # AV runs one iteration behind the scores/exp so the PE never
                # sem-waits on the scalar engine inside its queue
                prev = None
                for i in range(TT):
                    ps = pspool.tile([P, 1024], FP32, tag="ps", bufs=2,
                                     name=f"s{j}_{qh}_{i}")
                    # two heads of the pair: concurrent row-tiled matmuls
                    nc.tensor.matmul(
                        ps[:, 0:512],
                        lhsT=ktb[0:S, j, i * P:(i + 1) * P],
                        rhs=qtb[0:S, j, q0:q0 + QW],
                        start=True, stop=True)
                    nc.tensor.matmul(
                        ps[:, 512:1024],
                        lhsT=ktb[S:P, j, i * P:(i + 1) * P],
                        rhs=qtb[S:P, j, q0:q0 + QW],
                        start=True, stop=True)
                    pts = mp.tile([P, 1024], BF16, tag="pt", bufs=4,
                                  name=f"p{j}_{qh}_{i}")
                    nc.scalar.activation(pts[:], ps[:], AF.Exp)
                    if prev is not None:
                        emit_av(*prev)
                    prev = (i, pts)
                    # spread filler projections evenly over this pair's slots
                    for fn in fill.get(slot, ()):
                        fn()
                    slot += 1
                emit_av(*prev)
                # drain AV into bf16 staging; denominator row -> DRAM
                for par, av in ((0, avA), (1, avB)):
                    yraw = mp.tile([P, QW], BF16, tag="yraw", bufs=34,
                                   name=f"yraw{j}_{qh}_{par}")
                    nc.vector.tensor_copy(out=yraw[0:S + 1, :],
                                          in_=av[0:S + 1, :])
                    nc.sync.dma_start(dramd[qh][2 * j + par:2 * j + par + 1, :],
                                      yraw[S:S + 1, :])
                    yraws[(qh, j, par)] = yraw
            if j == FT - 1:
                for qh in range(NQH):
                    epilogue(qh)
            elif j == FT - 2:
                pass  # epilogues all run after the last pair


_NC = None


def _get_nc():
    global _NC
    if _NC is None:
        _NC = build_nc()
    return _NC


def make_in_maps(X, W_k, W_q, W_v, W_u, b_u):
    bf16 = ml_dtypes.bfloat16
    X = np.asarray(X, np.float32)
    b = X.shape[0]
    wkt = (np.asarray(W_k, np.float32).T * SCALE).astype(bf16)
    wqt = (np.asarray(W_q, np.float32).T * SCALE).astype(bf16)
    wvt = (np.asarray(W_v, np.float32).T * SCALE).astype(bf16)
    wut = np.ascontiguousarray(np.asarray(W_u, np.float32).T).astype(bf16)
    bu2 = np.ascontiguousarray(
        (np.asarray(b_u, np.float32) * 0.5).reshape(1, E))
    xts = [np.ascontiguousarray(X[bi].T).astype(bf16) for bi in range(b)]
    in_maps = []
    for c in range(N_CORES):
        bi, pg = c // 2, c % 2
        f0 = pg * F
        in_maps.append({
            "xt": xts[bi],
            "wk": np.ascontiguousarray(wkt[:, f0:f0 + F]),
            "wq": np.ascontiguousarray(wqt[:, f0:f0 + F]),
            "wv": np.ascontiguousarray(wvt[:, f0:f0 + F]),
            "wu": np.ascontiguousarray(wut[f0:f0 + F, :]),
            "bu": bu2,
        })
    return in_maps


def run(inputs, trace=False, **kwargs):
    """Run on hardware; returns (full output, BassKernelResults)."""
    X = np.asarray(inputs["X"], np.float32)
    b, t, e = X.shape
    nc = _get_nc()
    in_maps = make_in_maps(X, inputs["W_k"], inputs["W_q"], inputs["W_v"],
                           inputs["W_u"], inputs["b_u"])
    res = run_bass_kernel_spmd(nc, in_maps, core_ids=list(range(N_CORES)),
                               trace=trace, **kwargs)
    full = np.empty((b, t, e), np.float32)
    for bi in range(b):
        full[bi] = res.results[2 * bi]["out"] + res.results[2 * bi + 1]["out"]
    return full, res


def kernel(**inputs):
    full, _ = run(inputs)
    return full


# revision 13
# speedup vs baseline: 1.1873x; 1.0023x over previous
# Multi-head attention kernel for Trainium2 (8 NeuronCores, SPMD).
#
# Problem (hardcoded): X[4, 2048, 1024], W_k/W_q/W_v/W_u[1024, 1024], b_u[1024]
#   K = (X @ W_k.T) * s ; Q = (X @ W_q.T) * s ; V = (X @ W_v.T) * s   (s = 1024**-0.25)
#   S = Q @ K.T per head (16 heads, head_dim 64); P = softmax(S); Y = P @ V
#   out = Y @ W_u.T + b_u
#
# Sharding: core c handles (batch c//2, head-group c%2) -- 8 of the 16 heads.
# Each core computes K/Q/V projections only for its own 8 heads (512 of the
# 1024 features), runs attention for those heads over the full sequence, and
# produces a PARTIAL output projection out_c = (Y_c / D_c) @ W_u.T[gc] + b_u/2.
# The host unshard sums the two partial outputs per batch (pure reduction).
#
# Per-core layout (PE always contracts on partitions):
#   X^T   [e, t]        bf16 from host
#   K^T   [128, j, t]   pair-major: pair j's heads at partitions 0-63 / 64-127
#   Q^T   [128, j, t]   same; lets the two heads of a pair run as CONCURRENT
#                       row-tiled score matmuls (K=64 each, tile (0,0)+(64,0))
#   V     [t, i, h, 65] token-major, 65th column = ones (softmax denominator
#                       arrives free as row 64 of the P@V accumulation)
#   S^T   [tk, q]       scores transposed; exp is layout-agnostic and AV wants
#                       P with keys on partitions
#   Y^T   [128, kt, q]  normalized AV output, bf16, feeds the out-projection
#
# Scale s is folded into the host-side weight slices; all inputs arrive bf16.

import numpy as np
import ml_dtypes

import concourse.bacc as bacc
import concourse.mybir as mybir
import concourse.tile as tile
from concourse.bass_utils import run_bass_kernel_spmd

FP32 = mybir.dt.float32
BF16 = mybir.dt.bfloat16
AF = mybir.ActivationFunctionType

P = 128
E = 1024          # embedding dim
F = 512           # features per core (8 heads x 64)
H = 8             # heads per core
S = 64            # head dim
ET = E // P       # 8 contraction tiles over e
FT = F // P       # 4 feature tiles (= head pairs)
T = 2048          # sequence length
TT = T // P       # 16 key tiles
NQH = 4           # query quarters
QW = T // NQH     # 512 queries per quarter
SCALE = float(1024.0 ** -0.25)

N_CORES = 8


def build_nc():
    nc = bacc.Bacc("TRN2", target_bir_lowering=False, debug=False,
                   enable_asserts=False)

    xt = nc.dram_tensor("xt", [E, T], BF16, kind="ExternalInput").ap()
    wk = nc.dram_tensor("wk", [E, F], BF16, kind="ExternalInput").ap()
    wq = nc.dram_tensor("wq", [E, F], BF16, kind="ExternalInput").ap()
    wv = nc.dram_tensor("wv", [E, F], BF16, kind="ExternalInput").ap()
    wu = nc.dram_tensor("wu", [F, E], BF16, kind="ExternalInput").ap()
    bu = nc.dram_tensor("bu", [1, E], FP32, kind="ExternalInput").ap()
    out = nc.dram_tensor("out", [T, E], FP32, kind="ExternalOutput").ap()

    with tile.TileContext(nc) as tc:
        _build_kernel(tc, nc, xt, wk, wq, wv, wu, bu, out)
    nc.compile()
    return nc


def _build_kernel(tc, nc, xt, wk, wq, wv, wu, bu, out):
    with (
        tc.tile_pool(name="main", bufs=1) as mp,
        tc.tile_pool(name="psum", bufs=1, space="PSUM") as pspool,
        tc.tile_pool(name="dram", bufs=1, space="DRAM") as drampool,
    ):
        bub = mp.tile([P, E], FP32, tag="bub", name="bub")
        nc.sync.dma_start(bub[:], bu.to_broadcast([P, E]))

        # inputs, bf16, tiled for contraction on partitions
        xtb = mp.tile([P, ET, T], BF16, tag="xtb", name="xtb")
        wkb = mp.tile([P, ET, F], BF16, tag="wkb", name="wkb")
        wqb = mp.tile([P, ET, F], BF16, tag="wqb", name="wqb")
        wvb = mp.tile([P, ET, F], BF16, tag="wvb", name="wvb")
        for k in range(ET):
            nc.sync.dma_start(xtb[:, k, :], xt[k * P:(k + 1) * P, :])
            nc.sync.dma_start(wkb[:, k, :], wk[k * P:(k + 1) * P, :])
            nc.sync.dma_start(wqb[:, k, :], wq[k * P:(k + 1) * P, :])
            nc.sync.dma_start(wvb[:, k, :], wv[k * P:(k + 1) * P, :])

        ktb = mp.tile([P, FT, T], BF16, tag="ktb", name="ktb")
        qtb = mp.tile([P, FT, T], BF16, tag="qtb", name="qtb")
        vv = mp.tile([P, TT, H, S + 1], BF16, tag="vv", name="vv")
        yt = mp.tile([P, FT, T], BF16, tag="yt", name="yt")

        def kq_piece(j, wb, dst, c0, half, state):
            """Half of one 512-column K/Q projection chunk (4 of 8 k-tiles);
            the accumulation group spans both pieces so a piece fits in the
            attention loop's per-slot PE slack."""
            if half == 0:
                state[(j, id(wb), c0)] = pspool.tile(
                    [P, 1024], FP32, tag="ps", bufs=2,
                    name=f"pskq{j}_{id(wb)}_{c0}")
            ps = state[(j, id(wb), c0)]
            for k in range(half * 4, half * 4 + 4):
                nc.tensor.matmul(
                    ps[:, 0:512],
                    lhsT=wb[:, k, j * P:(j + 1) * P],
                    rhs=xtb[:, k, c0:c0 + 512],
                    start=(k == 0), stop=(k == ET - 1))
            if half == 1:
                del state[(j, id(wb), c0)]
                nc.vector.tensor_copy(out=dst[:, j, c0:c0 + 512],
                                      in_=ps[:, 0:512])

        def v_group(mt):
            """V projection for token tile mt -> vv[:, mt] + ones column."""
            ps = pspool.tile([P, 1024], FP32, tag="ps", bufs=2,
                             name=f"psv{mt}")
            for k in range(ET):
                nc.tensor.matmul(
                    ps[:, 0:512],
                    lhsT=xtb[:, k, mt * P:(mt + 1) * P],
                    rhs=wvb[:, k, :],
                    start=(k == 0), stop=(k == ET - 1))
            nc.vector.tensor_copy(
                out=vv[:, mt, :, 0:S],
                in_=ps[:, 0:512].rearrange("p (h s) -> p h s", s=S))
            nc.vector.memset(vv[:, mt, :, S:S + 1], 1.0)

        # K/Q projection of pair 0 first (fills the score pipeline), then a
        # head start on V so AV never outruns the V projection.
        _st = {}
        for c0 in range(0, T, 512):
            for wb, dst in ((wkb, ktb), (wqb, qtb)):
                kq_piece(0, wb, dst, c0, 0, _st)
                kq_piece(0, wb, dst, c0, 1, _st)
        for mt in range(3):
            v_group(mt)

        # wu shares wvb's slot (dead after V projection completes)
        wub = mp.tile([P, FT, E], BF16, tag="wvb", name="wub")

        def load_wu():
            nc.sync.dma_start(
                wub[:], wu.rearrange("(kt p) e -> p kt e", p=P))

        # per-(pair, slot) filler work, spread into the attention loop's PE
        # slack so the scalar engine (exp) stays the critical path. V tiles
        # must land at >= 1/slot during pair 0's first quarter (AV consumes
        # V(i) at slot i), so they occupy slots 0..12 back-to-back.
        sched = {j: {} for j in range(FT)}
        for idx, mt in enumerate(range(3, TT)):
            sched[0].setdefault(idx, []).append(lambda mt=mt: v_group(mt))

        kq_state = {}

        def kq_closures(nj):
            return [
                (lambda nj=nj, wb=wb, dst=dst, c0=c0, half=half:
                 kq_piece(nj, wb, dst, c0, half, kq_state))
                for wb, dst in ((wkb, ktb), (wqb, qtb))
                for c0 in range(0, T, 512)
                for half in (0, 1)
            ]

        rest0 = kq_closures(1) + [load_wu]
        for idx, fn in enumerate(rest0):
            s = 13 + (NQH * TT - 13) * idx // len(rest0)
            sched[0].setdefault(s, []).append(fn)
        for j in (1, 2):
            lst = kq_closures(j + 1)
            for idx, fn in enumerate(lst):
                sched[j].setdefault(NQH * TT * idx // len(lst), []).append(fn)

        # epilogue state: unnormalized Y (bf16) + denominators staged in DRAM
        yraws = {}
        dramd = {qh: drampool.tile([H, QW], BF16, tag=f"dD{qh}", bufs=1,
                                   name=f"dramD{qh}")
                 for qh in range(NQH)}

        def epilogue(qh):
            """Batched reciprocal of this quarter's 8 denominators, broadcast
            via DRAM bounce, normalize into yt, then the output projection."""
            q0 = qh * QW
            dsb = mp.tile([P, QW], BF16, tag="dsb", bufs=2, name=f"dsb{qh}")
            nc.sync.dma_start(dsb[0:H, :], dramd[qh][:])
            rcb = mp.tile([P, QW], FP32, tag="rcb", bufs=2, name=f"rcb{qh}")
            nc.vector.reciprocal(out=rcb[0:H, :], in_=dsb[0:H, :])
            dramr = drampool.tile([H, QW], FP32, tag="dR", bufs=2,
                                  name=f"dramR{qh}")
            nc.sync.dma_start(dramr[:], rcb[0:H, :])
            for j in range(FT):
                for par in range(2):
                    h = 2 * j + par
                    rbc = mp.tile([S, QW], FP32, tag="rbc", bufs=4,
                                  name=f"rbc{qh}_{h}")
                    nc.sync.dma_start(
                        rbc[:], dramr[h:h + 1, :].to_broadcast([S, QW]))
                    yraw = yraws.pop((qh, j, par))
                    if par == 0:
                        nc.vector.tensor_mul(out=yt[0:S, j, q0:q0 + QW],
                                             in0=yraw[0:S, :], in1=rbc[:])
                    else:
                        tmp = mp.tile([S, QW], BF16, tag="tmp", bufs=2,
                                      name=f"tmp{qh}_{h}")
                        nc.vector.tensor_mul(out=tmp[:], in0=yraw[0:S, :],
                                             in1=rbc[:])
                        nc.sync.dma_start(yt[S:P, j, q0:q0 + QW], tmp[:])
            for m in range(QW // P):
                ps = pspool.tile([P, 1024], FP32, tag="ps", bufs=2,
                                 name=f"pso{qh}_{m}")
                mc = q0 + m * P
                for n0 in (0, 512):
                    for kt in range(FT):
                        nc.tensor.matmul(
                            ps[:, n0:n0 + 512],
                            lhsT=yt[:, kt, mc:mc + P],
                            rhs=wub[:, kt, n0:n0 + 512],
                            start=(kt == 0), stop=(kt == FT - 1))
                ot = mp.tile([P, E], FP32, tag="ot", bufs=2,
                             name=f"ot{qh}_{m}")
                nc.vector.tensor_add(out=ot[:], in0=ps[:], in1=bub[:])
                nc.sync.dma_start(out[mc:mc + P, :], ot[:])

        # --- attention: pair-outer, query-quarter inner ---
        for j in range(FT):
            fill = sched[j]
            slot = 0
            for qh in range(NQH):
                q0 = qh * QW
                avA = pspool.tile([P, QW], FP32, tag="avA", bufs=2,
                                  name=f"av{j}_{qh}_0")
                avB = pspool.tile([P, QW], FP32, tag="avB", bufs=2,
                                  name=f"av{j}_{qh}_1")
                def emit_av(i, pts):
                    nc.tensor.matmul(
                        avA[0:S + 1, :],
                        lhsT=vv[:, i, 2 * j, :],
                        rhs=pts[:, 0:512],
                        start=(i == 0), stop=(i == TT - 1))
                    nc.tensor.matmul(
                        avB[0:S + 1, :],
                        lhsT=vv[:, i, 2 * j + 1, :],
                        rhs=pts[:, 512:1024],
                        start=(i == 0), stop=(i == TT - 1))

                # AV runs two iterations behind the scores/exp so the PE never
                # sem-waits on the scalar engine inside its queue
                pend = []
                for i in range(TT):
                    ps = pspool.tile([P, 1024], FP32, tag="ps", bufs=2,
                                     name=f"s{j}_{qh}_{i}")
                    # two heads of the pair: concurrent row-tiled matmuls
                    nc.tensor.matmul(
                        ps[:, 0:512],
                        lhsT=ktb[0:S, j, i * P:(i + 1) * P],
                        rhs=qtb[0:S, j, q0:q0 + QW],
                        start=True, stop=True)
                    nc.tensor.matmul(
                        ps[:, 512:1024],
                        lhsT=ktb[S:P, j, i * P:(i + 1) * P],
                        rhs=qtb[S:P, j, q0:q0 + QW],
                        start=True, stop=True)
                    pts = mp.tile([P, 1024], BF16, tag="pt", bufs=4,
                                  name=f"p{j}_{qh}_{i}")
                    nc.scalar.activation(pts[:], ps[:], AF.Exp)
                    pend.append((i, pts))
                    if len(pend) > 2:
                        emit_av(*pend.pop(0))
                    # spread filler projections evenly over this pair's slots
                    for fn in fill.get(slot, ()):
                        fn()
                    slot += 1
                for item in pend:
                    emit_av(*item)
                # drain AV into bf16 staging; denominator row -> DRAM
                for par, av in ((0, avA), (1, avB)):
                    yraw = mp.tile([P, QW], BF16, tag="yraw", bufs=34,
                                   name=f"yraw{j}_{qh}_{par}")
                    nc.vector.tensor_copy(out=yraw[0:S + 1, :],
                                          in_=av[0:S + 1, :])
                    nc.sync.dma_start(dramd[qh][2 * j + par:2 * j + par + 1, :],
                                      yraw[S:S + 1, :])
                    yraws[(qh, j, par)] = yraw
            if j == FT - 1:
                for qh in range(NQH):
                    epilogue(qh)
            elif j == FT - 2:
                pass  # epilogues all run after the last pair


_NC = None


def _get_nc():
    global _NC
    if _NC is None:
        _NC = build_nc()
    return _NC


def make_in_maps(X, W_k, W_q, W_v, W_u, b_u):
    bf16 = ml_dtypes.bfloat16
    X = np.asarray(X, np.float32)
    b = X.shape[0]
    wkt = (np.asarray(W_k, np.float32).T * SCALE).astype(bf16)
    wqt = (np.asarray(W_q, np.float32).T * SCALE).astype(bf16)
    wvt = (np.asarray(W_v, np.float32).T * SCALE).astype(bf16)
    wut = np.ascontiguousarray(np.asarray(W_u, np.float32).T).astype(bf16)
    bu2 = np.ascontiguousarray(
        (np.asarray(b_u, np.float32) * 0.5).reshape(1, E))
    xts = [np.ascontiguousarray(X[bi].T).astype(bf16) for bi in range(b)]
    in_maps = []
    for c in range(N_CORES):
        bi, pg = c // 2, c % 2
        f0 = pg * F
        in_maps.append({
            "xt": xts[bi],
            "wk": np.ascontiguousarray(wkt[:, f0:f0 + F]),
            "wq": np.ascontiguousarray(wqt[:, f0:f0 + F]),
            "wv": np.ascontiguousarray(wvt[:, f0:f0 + F]),
            "wu": np.ascontiguousarray(wut[f0:f0 + F, :]),
            "bu": bu2,
        })
    return in_maps


def run(inputs, trace=False, **kwargs):
    """Run on hardware; returns (full output, BassKernelResults)."""
    X = np.asarray(inputs["X"], np.float32)
    b, t, e = X.shape
    nc = _get_nc()
    in_maps = make_in_maps(X, inputs["W_k"], inputs["W_q"], inputs["W_v"],
                           inputs["W_u"], inputs["b_u"])
    res = run_bass_kernel_spmd(nc, in_maps, core_ids=list(range(N_CORES)),
                               trace=trace, **kwargs)
    full = np.empty((b, t, e), np.float32)
    for bi in range(b):
        full[bi] = res.results[2 * bi]["out"] + res.results[2 * bi + 1]["out"]
    return full, res


def kernel(**inputs):
    full, _ = run(inputs)
    return full


# revision 14
# speedup vs baseline: 1.2797x; 1.0778x over previous
# Multi-head attention kernel for Trainium2 (8 NeuronCores, SPMD).
#
# Problem (hardcoded): X[4, 2048, 1024], W_k/W_q/W_v/W_u[1024, 1024], b_u[1024]
#   K = (X @ W_k.T) * s ; Q = (X @ W_q.T) * s ; V = (X @ W_v.T) * s   (s = 1024**-0.25)
#   S = Q @ K.T per head (16 heads, head_dim 64); P = softmax(S); Y = P @ V
#   out = Y @ W_u.T + b_u
#
# Sharding: core c handles (batch c//2, head-group c%2) -- 8 of the 16 heads.
# Each core computes K/Q/V projections only for its own 8 heads (512 of the
# 1024 features), runs attention for those heads over the full sequence, and
# produces a PARTIAL output projection out_c = (Y_c / D_c) @ W_u.T[gc] + b_u/2.
# The host unshard sums the two partial outputs per batch (pure reduction).
#
# Per-core layout (PE always contracts on partitions):
#   X^T   [e, t]        bf16 from host
#   K^T   [128, j, t]   pair-major: pair j's heads at partitions 0-63 / 64-127
#   Q^T   [128, j, t]   same; lets the two heads of a pair run as CONCURRENT
#                       row-tiled score matmuls (K=64 each, tile (0,0)+(64,0))
#   V     [t, i, h, 65] token-major, 65th column = ones (softmax denominator
#                       arrives free as row 64 of the P@V accumulation)
#   S^T   [tk, q]       scores transposed; exp is layout-agnostic and AV wants
#                       P with keys on partitions
#   Y^T   [128, kt, q]  normalized AV output, bf16, feeds the out-projection
#
# Scale s is folded into the host-side weight slices; all inputs arrive bf16.

import numpy as np
import ml_dtypes

import concourse.bacc as bacc
import concourse.mybir as mybir
import concourse.tile as tile
from concourse.bass_utils import run_bass_kernel_spmd

FP32 = mybir.dt.float32
BF16 = mybir.dt.bfloat16
AF = mybir.ActivationFunctionType

P = 128
E = 1024          # embedding dim
F = 512           # features per core (8 heads x 64)
H = 8             # heads per core
S = 64            # head dim
ET = E // P       # 8 contraction tiles over e
FT = F // P       # 4 feature tiles (= head pairs)
T = 2048          # sequence length
TT = T // P       # 16 key tiles
NQH = 4           # query quarters
QW = T // NQH     # 512 queries per quarter
SCALE = float(1024.0 ** -0.25)

N_CORES = 8


def build_nc():
    nc = bacc.Bacc("TRN2", target_bir_lowering=False, debug=False,
                   enable_asserts=False)

    xt = nc.dram_tensor("xt", [E, T], BF16, kind="ExternalInput").ap()
    wk = nc.dram_tensor("wk", [E, F], BF16, kind="ExternalInput").ap()
    wq = nc.dram_tensor("wq", [E, F], BF16, kind="ExternalInput").ap()
    wv = nc.dram_tensor("wv", [E, F], BF16, kind="ExternalInput").ap()
    wu = nc.dram_tensor("wu", [F, E], BF16, kind="ExternalInput").ap()
    bu = nc.dram_tensor("bu", [1, E], FP32, kind="ExternalInput").ap()
    out = nc.dram_tensor("out", [T, E], FP32, kind="ExternalOutput").ap()

    with tile.TileContext(nc) as tc:
        _build_kernel(tc, nc, xt, wk, wq, wv, wu, bu, out)
    nc.compile()
    return nc


def _build_kernel(tc, nc, xt, wk, wq, wv, wu, bu, out):
    with (
        tc.tile_pool(name="main", bufs=1) as mp,
        tc.tile_pool(name="psum", bufs=1, space="PSUM") as pspool,
        tc.tile_pool(name="dram", bufs=1, space="DRAM") as drampool,
    ):
        bub = mp.tile([P, E], FP32, tag="bub", name="bub")
        nc.sync.dma_start(bub[:], bu.to_broadcast([P, E]))

        # inputs, bf16, tiled for contraction on partitions
        xtb = mp.tile([P, ET, T], BF16, tag="xtb", name="xtb")
        wkb = mp.tile([P, ET, F], BF16, tag="wkb", name="wkb")
        wqb = mp.tile([P, ET, F], BF16, tag="wqb", name="wqb")
        wvb = mp.tile([P, ET, F], BF16, tag="wvb", name="wvb")
        for k in range(ET):
            nc.sync.dma_start(xtb[:, k, :], xt[k * P:(k + 1) * P, :])
            nc.sync.dma_start(wkb[:, k, :], wk[k * P:(k + 1) * P, :])
            nc.sync.dma_start(wqb[:, k, :], wq[k * P:(k + 1) * P, :])
            nc.sync.dma_start(wvb[:, k, :], wv[k * P:(k + 1) * P, :])

        ktb = mp.tile([P, FT, T], BF16, tag="ktb", name="ktb")
        qtb = mp.tile([P, FT, T], BF16, tag="qtb", name="qtb")
        vv = mp.tile([P, TT, H, S + 1], BF16, tag="vv", name="vv")
        yt = mp.tile([P, FT, T], BF16, tag="yt", name="yt")

        def kq_piece(j, wb, dst, c0, half, state):
            """Half of one 512-column K/Q projection chunk (4 of 8 k-tiles);
            the accumulation group spans both pieces so a piece fits in the
            attention loop's per-slot PE slack."""
            if half == 0:
                state[(j, id(wb), c0)] = pspool.tile(
                    [P, 512], FP32, tag="fps", bufs=2,
                    name=f"pskq{j}_{id(wb)}_{c0}")
            ps = state[(j, id(wb), c0)]
            for k in range(half * 4, half * 4 + 4):
                nc.tensor.matmul(
                    ps[:],
                    lhsT=wb[:, k, j * P:(j + 1) * P],
                    rhs=xtb[:, k, c0:c0 + 512],
                    start=(k == 0), stop=(k == ET - 1))
            if half == 1:
                del state[(j, id(wb), c0)]
                nc.vector.tensor_copy(out=dst[:, j, c0:c0 + 512],
                                      in_=ps[:, 0:512])

        def v_group(mt):
            """V projection for token tile mt -> vv[:, mt] + ones column."""
            ps = pspool.tile([P, 512], FP32, tag="fps", bufs=2,
                             name=f"psv{mt}")
            for k in range(ET):
                nc.tensor.matmul(
                    ps[:],
                    lhsT=xtb[:, k, mt * P:(mt + 1) * P],
                    rhs=wvb[:, k, :],
                    start=(k == 0), stop=(k == ET - 1))
            nc.vector.tensor_copy(
                out=vv[:, mt, :, 0:S],
                in_=ps[:].rearrange("p (h s) -> p h s", s=S))
            nc.vector.memset(vv[:, mt, :, S:S + 1], 1.0)

        # K/Q projection of pair 0 first (fills the score pipeline), then a
        # head start on V so AV never outruns the V projection.
        _st = {}
        for c0 in range(0, T, 512):
            for wb, dst in ((wkb, ktb), (wqb, qtb)):
                kq_piece(0, wb, dst, c0, 0, _st)
                kq_piece(0, wb, dst, c0, 1, _st)
        for mt in range(3):
            v_group(mt)

        # wu shares wvb's slot (dead after V projection completes)
        wub = mp.tile([P, FT, E], BF16, tag="wvb", name="wub")

        def load_wu():
            nc.sync.dma_start(
                wub[:], wu.rearrange("(kt p) e -> p kt e", p=P))

        # per-(pair, slot) filler work, spread into the attention loop's PE
        # slack so the scalar engine (exp) stays the critical path. V tiles
        # must land at >= 1/slot during pair 0's first quarter (AV consumes
        # V(i) at slot i), so they occupy slots 0..12 back-to-back.
        sched = {j: {} for j in range(FT)}
        for idx, mt in enumerate(range(3, TT)):
            sched[0].setdefault(idx, []).append(lambda mt=mt: v_group(mt))

        kq_state = {}

        def kq_closures(nj):
            return [
                (lambda nj=nj, wb=wb, dst=dst, c0=c0, half=half:
                 kq_piece(nj, wb, dst, c0, half, kq_state))
                for wb, dst in ((wkb, ktb), (wqb, qtb))
                for c0 in range(0, T, 512)
                for half in (0, 1)
            ]

        rest0 = kq_closures(1) + [load_wu]
        for idx, fn in enumerate(rest0):
            s = 13 + (NQH * TT - 13) * idx // len(rest0)
            sched[0].setdefault(s, []).append(fn)
        for j in (1, 2):
            lst = kq_closures(j + 1)
            for idx, fn in enumerate(lst):
                sched[j].setdefault(NQH * TT * idx // len(lst), []).append(fn)

        # epilogue state: unnormalized Y (bf16) + denominators staged in DRAM
        yraws = {}
        dramd = {qh: drampool.tile([H, QW], BF16, tag=f"dD{qh}", bufs=1,
                                   name=f"dramD{qh}")
                 for qh in range(NQH)}

        def epilogue(qh):
            """Batched reciprocal of this quarter's 8 denominators, broadcast
            via DRAM bounce, normalize into yt, then the output projection."""
            q0 = qh * QW
            dsb = mp.tile([P, QW], BF16, tag="dsb", bufs=2, name=f"dsb{qh}")
            nc.sync.dma_start(dsb[0:H, :], dramd[qh][:])
            rcb = mp.tile([P, QW], FP32, tag="rcb", bufs=2, name=f"rcb{qh}")
            nc.vector.reciprocal(out=rcb[0:H, :], in_=dsb[0:H, :])
            dramr = drampool.tile([H, QW], FP32, tag="dR", bufs=2,
                                  name=f"dramR{qh}")
            nc.sync.dma_start(dramr[:], rcb[0:H, :])
            for j in range(FT):
                for par in range(2):
                    h = 2 * j + par
                    rbc = mp.tile([S, QW], FP32, tag="rbc", bufs=4,
                                  name=f"rbc{qh}_{h}")
                    nc.sync.dma_start(
                        rbc[:], dramr[h:h + 1, :].to_broadcast([S, QW]))
                    yraw = yraws.pop((qh, j, par))
                    if par == 0:
                        nc.vector.tensor_mul(out=yt[0:S, j, q0:q0 + QW],
                                             in0=yraw[0:S, :], in1=rbc[:])
                    else:
                        tmp = mp.tile([S, QW], BF16, tag="tmp", bufs=2,
                                      name=f"tmp{qh}_{h}")
                        nc.vector.tensor_mul(out=tmp[:], in0=yraw[0:S, :],
                                             in1=rbc[:])
                        nc.sync.dma_start(yt[S:P, j, q0:q0 + QW], tmp[:])
            for m in range(QW // P):
                mc = q0 + m * P
                for n0 in (0, 512):
                    ps = pspool.tile([P, 512], FP32, tag="fps", bufs=2,
                                     name=f"pso{qh}_{m}_{n0}")
                    for kt in range(FT):
                        nc.tensor.matmul(
                            ps[:],
                            lhsT=yt[:, kt, mc:mc + P],
                            rhs=wub[:, kt, n0:n0 + 512],
                            start=(kt == 0), stop=(kt == FT - 1))
                    ot = mp.tile([P, 512], FP32, tag="ot", bufs=4,
                                 name=f"ot{qh}_{m}_{n0}")
                    nc.vector.tensor_add(out=ot[:], in0=ps[:],
                                         in1=bub[:, n0:n0 + 512])
                    nc.sync.dma_start(out[mc:mc + P, n0:n0 + 512], ot[:])

        # --- attention: pair-outer, query-quarter inner ---
        for j in range(FT):
            fill = sched[j]
            slot = 0
            for qh in range(NQH):
                q0 = qh * QW
                avA = pspool.tile([P, QW], FP32, tag="avA", bufs=1,
                                  name=f"av{j}_{qh}_0")
                avB = pspool.tile([P, QW], FP32, tag="avB", bufs=1,
                                  name=f"av{j}_{qh}_1")
                def emit_av(i, pts):
                    nc.tensor.matmul(
                        avA[0:S + 1, :],
                        lhsT=vv[:, i, 2 * j, :],
                        rhs=pts[:, 0:512],
                        start=(i == 0), stop=(i == TT - 1))
                    nc.tensor.matmul(
                        avB[0:S + 1, :],
                        lhsT=vv[:, i, 2 * j + 1, :],
                        rhs=pts[:, 512:1024],
                        start=(i == 0), stop=(i == TT - 1))

                # AV runs two iterations behind the scores/exp so the PE never
                # sem-waits on the scalar engine inside its queue
                pend = []
                for i in range(TT):
                    ps = pspool.tile([P, 1024], FP32, tag="ps", bufs=2,
                                     name=f"s{j}_{qh}_{i}")
                    # two heads of the pair: concurrent row-tiled matmuls
                    nc.tensor.matmul(
                        ps[:, 0:512],
                        lhsT=ktb[0:S, j, i * P:(i + 1) * P],
                        rhs=qtb[0:S, j, q0:q0 + QW],
                        start=True, stop=True)
                    nc.tensor.matmul(
                        ps[:, 512:1024],
                        lhsT=ktb[S:P, j, i * P:(i + 1) * P],
                        rhs=qtb[S:P, j, q0:q0 + QW],
                        start=True, stop=True)
                    pts = mp.tile([P, 1024], BF16, tag="pt", bufs=4,
                                  name=f"p{j}_{qh}_{i}")
                    nc.scalar.activation(pts[:], ps[:], AF.Exp)
                    pend.append((i, pts))
                    if len(pend) > 2:
                        emit_av(*pend.pop(0))
                    # spread filler projections evenly over this pair's slots
                    for fn in fill.get(slot, ()):
                        fn()
                    slot += 1
                for item in pend:
                    emit_av(*item)
                # drain AV into bf16 staging; denominator row -> DRAM
                for par, av in ((0, avA), (1, avB)):
                    yraw = mp.tile([P, QW], BF16, tag="yraw", bufs=34,
                                   name=f"yraw{j}_{qh}_{par}")
                    nc.vector.tensor_copy(out=yraw[0:S + 1, :],
                                          in_=av[0:S + 1, :])
                    nc.sync.dma_start(dramd[qh][2 * j + par:2 * j + par + 1, :],
                                      yraw[S:S + 1, :])
                    yraws[(qh, j, par)] = yraw
            if j == FT - 1:
                for qh in range(NQH):
                    epilogue(qh)
            elif j == FT - 2:
                pass  # epilogues all run after the last pair


_NC = None


def _get_nc():
    global _NC
    if _NC is None:
        _NC = build_nc()
    return _NC


def make_in_maps(X, W_k, W_q, W_v, W_u, b_u):
    bf16 = ml_dtypes.bfloat16
    X = np.asarray(X, np.float32)
    b = X.shape[0]
    wkt = (np.asarray(W_k, np.float32).T * SCALE).astype(bf16)
    wqt = (np.asarray(W_q, np.float32).T * SCALE).astype(bf16)
    wvt = (np.asarray(W_v, np.float32).T * SCALE).astype(bf16)
    wut = np.ascontiguousarray(np.asarray(W_u, np.float32).T).astype(bf16)
    bu2 = np.ascontiguousarray(
        (np.asarray(b_u, np.float32) * 0.5).reshape(1, E))
    xts = [np.ascontiguousarray(X[bi].T).astype(bf16) for bi in range(b)]
    in_maps = []
    for c in range(N_CORES):
        bi, pg = c // 2, c % 2
        f0 = pg * F
        in_maps.append({
            "xt": xts[bi],
            "wk": np.ascontiguousarray(wkt[:, f0:f0 + F]),
            "wq": np.ascontiguousarray(wqt[:, f0:f0 + F]),
            "wv": np.ascontiguousarray(wvt[:, f0:f0 + F]),
            "wu": np.ascontiguousarray(wut[f0:f0 + F, :]),
            "bu": bu2,
        })
    return in_maps


def run(inputs, trace=False, **kwargs):
    """Run on hardware; returns (full output, BassKernelResults)."""
    X = np.asarray(inputs["X"], np.float32)
    b, t, e = X.shape
    nc = _get_nc()
    in_maps = make_in_maps(X, inputs["W_k"], inputs["W_q"], inputs["W_v"],
                           inputs["W_u"], inputs["b_u"])
    res = run_bass_kernel_spmd(nc, in_maps, core_ids=list(range(N_CORES)),
                               trace=trace, **kwargs)
    full = np.empty((b, t, e), np.float32)
    for bi in range(b):
        full[bi] = res.results[2 * bi]["out"] + res.results[2 * bi + 1]["out"]
    return full, res


def kernel(**inputs):
    full, _ = run(inputs)
    return full
